# revision 17
# baseline (speedup 1.0000x reference)
"""Trainium2 Bass kernel for nn_DiffEqSolver (RK4 odeint of a 2-layer tanh MLP).

reference:  dz/dt = tanh(z @ W1 + b1) @ W2 + b2, classical RK4 over time grid t,
            returns trajectory [T, B, D] with traj[0] == z0.

Strategy (8 NeuronCores, data-parallel over batch):
  - Each core owns a 128-row batch shard (B=1024 -> 8 x 128).
  - Activations live TRANSPOSED on chip: z^T is [D=512, Bs=128], stored as an
    SBUF tile [128, 512] whose column block c holds (d-chunk c) x batch.
    With this layout BOTH matmuls use the natural weight layouts as the
    stationary operand (lhsT) and no on-chip transpose is ever needed:
      a^T[h,b] = sum_c W1[c-chunk, h-chunk].T @ y^T[c-chunk]   (lhsT = W1 slice)
      f^T[d,b] = sum_j W2[j-chunk, d-chunk].T @ tanh^T[j-chunk] (lhsT = W2 slice)
  - Matmuls run in bf16 (fp32 PSUM accumulate); RK4 state math stays fp32 on
    the vector engine. Measured end-to-end trajectory error vs the fp32
    reference is ~1e-3 relative.
  - tanh + PSUM->SBUF eviction fused on the scalar (ACT) engine.
  - Biases (zero in practice) are folded in as K=1 rank-1 matmuls when nonzero.
  - The time loop is fully unrolled; dt values are baked as immediates.

Output is written in the transposed on-chip layout and unscrambled on host.
"""

import sys

sys.path.insert(0, "/opt/trn_rl_repo")

import numpy as np
import ml_dtypes

import concourse.bacc as bacc
import concourse.mybir as mybir
from concourse.tile import TileContext, add_dep_helper
from concourse.bass_utils import run_bass_kernel_spmd

N_CORES = 8
B, D, H = 1024, 512, 1024
BS = B // N_CORES  # 128 batch rows per core
DC = D // 128  # 4 d-chunks
HC = H // 128  # 8 h-chunks

F32 = mybir.dt.float32
BF16 = mybir.dt.bfloat16
MULT = None  # set lazily (mybir.AluOpType.mult)
ADD = None

_program_cache = {}


def _build_program(nsteps, dts, has_b1, has_b2):
    """Emit + compile the Bass program. dts: python list of fp32 dt values."""
    alu = mybir.AluOpType
    nc = bacc.Bacc("TRN2", target_bir_lowering=False, debug=False)

    w1d = nc.dram_tensor("w1", [D, H], BF16, kind="ExternalInput").ap()
    w2d = nc.dram_tensor("w2", [H, D], BF16, kind="ExternalInput").ap()
    z032d = nc.dram_tensor("z0t32", [128, D], F32, kind="ExternalInput").ap()
    z016d = nc.dram_tensor("z0t16", [128, D], BF16, kind="ExternalInput").ap()
    if has_b1:
        b1d = nc.dram_tensor("b1row", [1, H], BF16, kind="ExternalInput").ap()
    if has_b2:
        b2d = nc.dram_tensor("b2row", [1, D], BF16, kind="ExternalInput").ap()
    if has_b1 or has_b2:
        onesd = nc.dram_tensor("onesrow", [1, BS], BF16, kind="ExternalInput").ap()
    trajd = nc.dram_tensor("traj", [nsteps, 128, D], F32, kind="ExternalOutput").ap()

    with TileContext(nc) as tc:
        with (
            tc.tile_pool(name="const", bufs=1) as cpool,
            tc.tile_pool(name="state", bufs=3) as spool,
            tc.tile_pool(name="psum", bufs=2, space="PSUM") as ppool,
        ):
            # ---- one-time loads (split per chunk so the first matmuls
            # only wait for the slice they read) ----------------------------
            # w1s column block c (cols [c*H,(c+1)*H)) = W1[c*128:(c+1)*128, :]
            w1s = cpool.tile([128, DC * H], BF16, tag="w1s")
            for c in range(DC):
                nc.sync.dma_start(
                    out=w1s[:, c * H : (c + 1) * H],
                    in_=w1d[c * 128 : (c + 1) * 128, :],
                )
            # w2s column block j (cols [j*D,(j+1)*D)) = W2[j*128:(j+1)*128, :]
            w2s = cpool.tile([128, HC * D], BF16, tag="w2s")
            for j in range(HC):
                nc.sync.dma_start(
                    out=w2s[:, j * D : (j + 1) * D],
                    in_=w2d[j * 128 : (j + 1) * 128, :],
                )
            z32 = spool.tile([128, D], F32, tag="z32")
            nc.sync.dma_start(out=z32[:, :], in_=z032d[:, :])
            zb = spool.tile([128, D], BF16, tag="zb")
            nc.sync.dma_start(out=zb[:, :], in_=z016d[:, :])
            if has_b1:
                b1t = cpool.tile([1, H], BF16, tag="b1t")
                nc.sync.dma_start(out=b1t[:, :], in_=b1d[:, :])
            if has_b2:
                b2t = cpool.tile([1, D], BF16, tag="b2t")
                nc.sync.dma_start(out=b2t[:, :], in_=b2d[:, :])
            if has_b1 or has_b2:
                ones = cpool.tile([1, BS], BF16, tag="ones")
                nc.sync.dma_start(out=ones[:, :], in_=onesd[:, :])

            # ---- time loop (fully unrolled) -------------------------------
            # PSUM budget: pa0/pa1 (1 bank each) + pfA/pfB (1 bank each),
            # bufs=2 -> exactly 8 banks.
            #
            # PSUM semantics: start=True clears has_written for the WHOLE
            # bank, so exactly one start per bank-tile (its first matmul);
            # later matmuls first-touch-write / accumulate per element.
            for step in range(nsteps):
                dt = float(dts[step])
                ycoef = [0.5 * dt, 0.5 * dt, dt]  # y_{i+1} = z + c_i * k_i
                acc = spool.tile([128, D], F32, tag="acc")
                u = None
                src = zb
                for s in range(4):
                    # ---- MM1: a^T[h=j*128+p, b] ---------------------------
                    # pa split into three tiles that complete progressively
                    # (j0-3 at half-MM1, j4-5 at 3/4, j6-7 at end) so tanh
                    # evictions are emitted -- and semaphore-gated -- as
                    # early as possible.  MM2 consumes hT_j at ~110ns per j,
                    # so the ACT chain [512]+[256]+[256] just keeps ahead.
                    # pa0's c-groups of 4 MMs pace the yb consumption to
                    # match the DVE production rate of the previous stage.
                    hT = spool.tile([128, H], BF16, tag="hT")
                    pa0 = ppool.tile([128, 512], F32, tag="pa0", name="pa0", bufs=1)
                    pa1a = ppool.tile([128, 384], F32, tag="pa1a", name="pa1a", bufs=1)
                    pa1b = ppool.tile([128, 128], F32, tag="pa1b", name="pa1b", bufs=1)
                    prev_last_mm = None
                    for pa, jlo, nj in ((pa0, 0, 4), (pa1a, 4, 3), (pa1b, 7, 1)):
                        first_mm = None
                        if has_b1:
                            for jj in range(nj):
                                mm = nc.tensor.matmul(
                                    pa[:, jj * 128 : (jj + 1) * 128],
                                    lhsT=b1t[:, (jlo + jj) * 128 : (jlo + jj + 1) * 128],
                                    rhs=ones[:, :],
                                    start=(jj == 0),
                                    stop=False,
                                )
                                first_mm = first_mm or mm
                        for c in range(DC):
                            for jj in range(nj):
                                j = jlo + jj
                                mm = nc.tensor.matmul(
                                    pa[:, jj * 128 : (jj + 1) * 128],
                                    lhsT=w1s[:, c * H + j * 128 : c * H + (j + 1) * 128],
                                    rhs=src[:, c * 128 : (c + 1) * 128],
                                    start=(c == 0 and jj == 0 and not has_b1),
                                    stop=(c == DC - 1 and jj == nj - 1),
                                )
                                first_mm = first_mm or mm
                        # order-only edge: keep the scheduler from hoisting
                        # this tile's matmuls ahead of the previous tile's
                        # tail (same-bank pairs are already serialized within
                        # a tile), so each pa tile -- and therefore its tanh's
                        # semaphore threshold -- completes as early as the
                        # dataflow allows.
                        if prev_last_mm is not None:
                            add_dep_helper(
                                first_mm.ins, prev_last_mm.ins, sync=False,
                                reason="sequence pa tiles",
                            )
                        prev_last_mm = mm
                        # tanh eviction emitted immediately after its pa tile
                        nc.scalar.activation(
                            hT[:, jlo * 128 : (jlo + nj) * 128],
                            pa[:, :],
                            mybir.ActivationFunctionType.Tanh,
                        )
                    # ---- MM2: f^T[d=c*128+p, b] ---------------------------
                    # Three tiles completing progressively: pfA (c=0,1,
                    # pair-sweep over j) at half-MM2, then c-solo sweeps pf2
                    # and pf3 at 3/4 and end.  Each tile's RK4 combines are
                    # emitted right after it, so yb_c / zbn_c chunks arrive
                    # just ahead of the next MM1's c-group consumption.
                    pfA = ppool.tile([128, 256], F32, tag="pfA", name="pfA", bufs=1)
                    pf2 = ppool.tile([128, 128], F32, tag="pf2", name="pf2", bufs=1)
                    pf3 = ppool.tile([128, 128], F32, tag="pf3", name="pf3", bufs=1)
                    if s < 3:
                        ybn = spool.tile([128, D], BF16, tag="yb")
                        out16, c16, in16 = ybn, ycoef[s], z32
                    else:
                        z32n = spool.tile([128, D], F32, tag="z32")
                        zbn = spool.tile([128, D], BF16, tag="zb")
                        out16, c16, in16 = zbn, dt / 6.0, u

                    def combines(pf, clo, ncols):
                        # bf16 chunks only -- the next MM1's critical path.
                        for ci in range(ncols):
                            cs = slice((clo + ci) * 128, (clo + ci + 1) * 128)
                            nc.vector.scalar_tensor_tensor(
                                out16[:, cs], pf[:, ci * 128 : (ci + 1) * 128],
                                c16, in16[:, cs], alu.mult, alu.add,
                            )

                    def fp32_update(pf, clo, ncols):
                        # accumulator / state update, deferred off the
                        # critical path.
                        rng = slice(clo * 128, (clo + ncols) * 128)
                        if s < 3:
                            if s == 0:
                                nc.vector.tensor_scalar_mul(acc[:, rng], pf[:, :], 1.0)
                            else:
                                nc.vector.scalar_tensor_tensor(
                                    acc[:, rng], pf[:, :], 2.0, acc[:, rng],
                                    alu.mult, alu.add,
                                )
                        else:
                            nc.vector.scalar_tensor_tensor(
                                z32n[:, rng], pf[:, :], dt / 6.0, u[:, rng],
                                alu.mult, alu.add,
                            )

                    for pf, clo, ncols in ((pfA, 0, 2), (pf2, 2, 1), (pf3, 3, 1)):
                        first_mm = None
                        if has_b2:
                            for ci in range(ncols):
                                mm = nc.tensor.matmul(
                                    pf[:, ci * 128 : (ci + 1) * 128],
                                    lhsT=b2t[:, (clo + ci) * 128 : (clo + ci + 1) * 128],
                                    rhs=ones[:, :],
                                    start=(ci == 0),
                                    stop=False,
                                )
                                first_mm = first_mm or mm
                        for j in range(HC):
                            for ci in range(ncols):
                                c = clo + ci
                                mm = nc.tensor.matmul(
                                    pf[:, ci * 128 : (ci + 1) * 128],
                                    lhsT=w2s[:, j * D + c * 128 : j * D + (c + 1) * 128],
                                    rhs=hT[:, j * 128 : (j + 1) * 128],
                                    start=(j == 0 and ci == 0 and not has_b2),
                                    stop=(j == HC - 1 and ci == ncols - 1),
                                )
                                first_mm = first_mm or mm
                        if prev_last_mm is not None:
                            add_dep_helper(
                                first_mm.ins, prev_last_mm.ins, sync=False,
                                reason="sequence pf tiles",
                            )
                        prev_last_mm = mm
                        combines(pf, clo, ncols)
                    for pf, clo, ncols in ((pfA, 0, 2), (pf2, 2, 1), (pf3, 3, 1)):
                        fp32_update(pf, clo, ncols)
                    if s == 2:
                        # u = z + dt/6*(k1+2k2+2k3); then z_new = u + dt/6*k4
                        u = spool.tile([128, D], F32, tag="u")
                        nc.vector.scalar_tensor_tensor(
                            u[:, :], acc[:, :], dt / 6.0, z32[:, :],
                            alu.mult, alu.add,
                        )
                    if s == 3:
                        nc.sync.dma_start(out=trajd[step], in_=z32n[:, :])
                        z32, zb = z32n, zbn
                    else:
                        src = ybn

    nc.compile()
    return nc


def _get_program(nsteps, dts, has_b1, has_b2):
    key = (nsteps, bytes(np.asarray(dts, np.float32)), has_b1, has_b2)
    if key not in _program_cache:
        _program_cache[key] = _build_program(nsteps, dts, has_b1, has_b2)
    return _program_cache[key]


def _scramble(z):  # [128, D] natural -> transposed/scrambled on-chip layout
    return np.ascontiguousarray(
        z.T.reshape(DC, 128, 128).transpose(1, 0, 2).reshape(128, D)
    )


def _unscramble(o):  # [nsteps, 128, D] on-chip layout -> natural [nsteps, 128, D]
    return o.reshape(-1, 128, DC, 128).transpose(0, 3, 2, 1).reshape(-1, 128, D)


def run_kernel(z0, t, W1, b1, W2, b2, trace=False, tmpdir=None):
    z0 = np.asarray(z0, np.float32)
    t = np.asarray(t, np.float32)
    W1 = np.asarray(W1, np.float32)
    b1 = np.asarray(b1, np.float32)
    W2 = np.asarray(W2, np.float32)
    b2 = np.asarray(b2, np.float32)
    T = t.shape[0]
    nsteps = T - 1
    dts = np.diff(t).astype(np.float32)
    has_b1 = bool(np.any(b1))
    has_b2 = bool(np.any(b2))

    nc = _get_program(nsteps, dts, has_b1, has_b2)

    bf = ml_dtypes.bfloat16
    w1b = W1.astype(bf)
    w2b = W2.astype(bf)
    in_maps = []
    for s in range(N_CORES):
        zt = _scramble(z0[s * BS : (s + 1) * BS])
        m = {
            "w1": w1b,
            "w2": w2b,
            "z0t32": zt,
            "z0t16": zt.astype(bf),
        }
        if has_b1:
            m["b1row"] = b1.reshape(1, H).astype(bf)
        if has_b2:
            m["b2row"] = b2.reshape(1, D).astype(bf)
        if has_b1 or has_b2:
            m["onesrow"] = np.ones((1, BS), bf)
        in_maps.append(m)

    res = run_bass_kernel_spmd(
        nc, in_maps, list(range(N_CORES)), trace=trace, tmpdir=tmpdir
    )

    out = np.empty((T, B, D), np.float32)
    out[0] = z0
    for s in range(N_CORES):
        out[1:, s * BS : (s + 1) * BS] = _unscramble(res.results[s]["traj"])
    return out, res


def kernel(z0, t, W1, b1, W2, b2):
    out, _ = run_kernel(z0, t, W1, b1, W2, b2, trace=False)
    return out
